# revision 43
# baseline (speedup 1.0000x reference)
"""Trainium2 Bass kernel for BlockRecurrentAttention (causal attention w/ partial RoPE).

Sharding: 16 heads / 8 cores = 2 heads per core (tensor-parallel over heads).
Each core: QKV projection for its 128 W-columns, causal attention for its
2 heads x 2 batches, partial output projection (row-sharded Wout).
Host: sums the 8 partial outputs (the "all-reduce").

v2 design (vs baseline; cold-dispatch sim 248us -> 146us):
  - all matmuls bf16 (1.0 cycles/row at any width).
  - RoPE rotate-half via a PE matmul with a constant +-1 permutation matrix
    (pmat) instead of SBUF->SBUF shuffle DMAs; cos/sin applied by DVE.
  - PV in [q, dims] layout: lhsT = att^T block [kt, q], rhs = v block
    [kt, 65] (64 dims + ones column -> denominator lands in col 64).
    Halves PV PE cost vs the [dims, q] layout with replicated ones rows.
    The 8 per-(head, q-block) chains share a psum bank via the pending-zero
    write-through semantics (skip_group_check).
  - V computed directly in [tok, col] layout (xt chunk stationary) -- no PE
    transposes for V.
  - epilogue per 2-q-block half, emitted as soon as those PV chains finish:
    reciprocal of denom col + broadcast multiply, bf16 PE transpose of o to
    [d, q], out-projection, store -- overlaps the remaining attention.
  - causal masking multiplies only the diagonal 128x128 block by a 0/1
    triangle (gpsimd; engine choice balances DVE/ACT/Pool load).
  - few, large DMAs in a hand-ordered stream (all DMAs serialize on a
    shared resource in emission order; each costs ~625ns issue overhead):
    x tiles prefetch ahead of the tables their consumers can wait on,
    stores trail. PSUM-draining copies avoid gpsimd (PSUM-blind on HW).
  - software-pipelined attention: PV(kb) emitted after S(kb+1), exp on ACT
    rides one block behind the PE.
"""

import numpy as np

B, N, DIM, H, D, L = 2, 2048, 1024, 16, 64, 32
NCORES = 8
CPC = 128            # W columns per core (2 heads x 64)
T = B * N            # 4096 tokens, batch-major
SCALE = D ** -0.5
KI = 8               # contraction chunks of 128
TT = 512             # token tile
NTT = T // TT        # 8
QT = 512             # q tile in attention
NQT = N // QT        # 4 per batch
NKB = T // 128       # 32 token blocks

# blob column offsets (bf16 table pack)
O_COSP = 0
O_SINP = 2048
O_COSN = 4096        # [16, 32] flattened
O_SINN = 4608
O_MASK = 5120        # [128] causal triangle (c >= p)
O_WOUT = 5248
BLOB_COLS = 6272

_CACHE = {}


def _build_program(reps=1):
    import concourse.bacc as bacc
    import concourse.mybir as mybir
    import concourse.tile as tile
    from concourse.masks import make_identity
    from contextlib import ExitStack

    F32 = mybir.dt.float32
    BF16 = mybir.dt.bfloat16
    EXP = mybir.ActivationFunctionType.Exp

    class _ScalarCopy:
        def __init__(self, nc):
            self._nc = nc

        def tensor_copy(self, out, in_):
            self._nc.scalar.copy(out, in_)

    nc = bacc.Bacc("TRN2", target_bir_lowering=False, debug=False,
                   num_devices=NCORES, enable_partition_id=False)

    x_r = nc.dram_tensor("x_r", [128, KI, T], BF16, kind="ExternalInput").ap()
    wq_t = nc.dram_tensor("wq_t", [128, KI, 128], BF16, kind="ExternalInput").ap()
    wkv = nc.dram_tensor("wkv", [128, 2, KI, 128], BF16, kind="ExternalInput").ap()
    pmat = nc.dram_tensor("pmat", [128, 128], BF16, kind="ExternalInput").ap()
    blob = nc.dram_tensor("blob", [128, BLOB_COLS], BF16, kind="ExternalInput").ap()
    out = nc.dram_tensor("out", [T, DIM], BF16, kind="ExternalOutput").ap()

    with tile.TileContext(nc) as tc, ExitStack() as ctx:
        singles = ctx.enter_context(tc.tile_pool(name="singles", bufs=1))

        qT = singles.tile([128, T], BF16)          # 2 heads x 64 dims on partitions
        kT = singles.tile([128, T], BF16)
        # per 128-token block: [vA(64) | onesA(1) | vB(64) | onesB(1)]
        vsb = singles.tile([128, NKB, 130], BF16)
        wcat_sb = singles.tile([128, 3, KI, 128], BF16)
        pmat_sb = singles.tile([128, 128], BF16)
        blob_sb = singles.tile([128, BLOB_COLS], BF16)
        ident = singles.tile([128, 128], BF16)

        cosP = blob_sb[:, O_COSP:O_COSP + 2048]
        sinP = blob_sb[:, O_SINP:O_SINP + 2048]
        cosN = blob_sb[:, O_COSN:O_COSN + 512].rearrange("p (kb j) -> p kb j", j=L)
        sinN = blob_sb[:, O_SINN:O_SINN + 512].rearrange("p (kb j) -> p kb j", j=L)
        tri = blob_sb[:, O_MASK:O_MASK + 128]
        wout_sb = blob_sb[:, O_WOUT:O_WOUT + DIM]

        bigp = ctx.enter_context(tc.tile_pool(name="big", bufs=NTT))
        tmpp = ctx.enter_context(tc.tile_pool(name="tmp", bufs=2))
        vrp = ctx.enter_context(tc.tile_pool(name="vr", bufs=2))
        attp = ctx.enter_context(tc.tile_pool(name="att", bufs=4))
        onp = ctx.enter_context(tc.tile_pool(name="on", bufs=2))
        otp = ctx.enter_context(tc.tile_pool(name="ot", bufs=3))
        fop = ctx.enter_context(tc.tile_pool(name="fo", bufs=2))
        rcp = ctx.enter_context(tc.tile_pool(name="rc", bufs=2))

        # PSUM: psst 2x2 banks + pv 2x1 + pscr 2x1 = 8 banks
        psst = ctx.enter_context(tc.tile_pool(name="psst", bufs=2, space="PSUM"))
        pvp = ctx.enter_context(tc.tile_pool(name="pvp", bufs=1, space="PSUM"))
        pscr = ctx.enter_context(tc.tile_pool(name="pscr", bufs=2, space="PSUM"))

        def emit_qkv_tile(tt, xt):
            ts = tt * TT
            tloc = ts % N
            # q, k projections -> [dims, tok] bf16 (Pool does psum->sbuf copies)
            for wi, dst in ((0, qT), (1, kT)):
                ps = pscr.tile([128, TT], F32, tag="scr", name=f"ps{tt}_{wi}")
                for ki in range(KI):
                    nc.tensor.matmul(ps[:], wcat_sb[:, wi, ki, :],
                                     xt[:, ki, :], start=(ki == 0), stop=(ki == KI - 1))
                nc.vector.tensor_copy(dst[:, ts:ts + TT], ps[:])
            # v direct in [tok, col] layout: 4 token-blocks accumulate in one
            # psum bank (write-through zero region), one DVE copy drains it
            psv = pscr.tile([128, 4, 128], F32, tag="scr", name=f"psv{tt}")
            for j in range(TT // 128):
                for ki in range(KI):
                    nc.tensor.matmul(psv[:, j, :], xt[:, ki, j * 128:(j + 1) * 128],
                                     wcat_sb[:, 2, ki, :],
                                     start=(j == 0 and ki == 0),
                                     stop=(j == 3 and ki == KI - 1),
                                     skip_group_check=True)
            nc.vector.tensor_copy(
                vsb[:, tt * 4:tt * 4 + 4, :]
                .rearrange("p kb (h c) -> p kb h c", h=2, c=65)[:, :, :, 0:64],
                psv[:].rearrange("p kb (h c) -> p kb h c", h=2, c=64))
            # rope: rot = pmat @ t (PE), then t = t*cos + rot*sin (DVE)
            for src in (qT, kT):
                psr = pscr.tile([128, TT], F32, tag="scr", name=f"psr{tt}")
                nc.tensor.matmul(psr[:], pmat_sb[:], src[:, ts:ts + TT],
                                 start=True, stop=True)
                tq = tmpp.tile([128, TT], BF16, tag="tmp", name=f"tq{tt}")
                nc.vector.tensor_mul(tq[:], psr[:], sinP[:, tloc:tloc + TT])
                nc.vector.tensor_mul(src[:, ts:ts + TT], src[:, ts:ts + TT],
                                     cosP[:, tloc:tloc + TT])
                nc.vector.tensor_add(src[:, ts:ts + TT], src[:, ts:ts + TT], tq[:])
            # v rope (DVE, bf16 2x) for this tile's 4 blocks
            kb0 = tt * 4
            kbt = kb0 % 16
            vtmp = vrp.tile([128, 4, L], BF16, tag="vr", name=f"vtmp{tt}")
            for g0 in (0, 65):
                vh = vsb[:, kb0:kb0 + 4, g0:g0 + L]
                cN = cosN[:, kbt:kbt + 4, :]
                sN = sinN[:, kbt:kbt + 4, :]
                nc.gpsimd.tensor_mul(vtmp[:, :, 0:16], vh[:, :, 16:32], sN[:, :, 0:16])
                nc.gpsimd.tensor_mul(vtmp[:, :, 16:32], vh[:, :, 0:16], sN[:, :, 16:32])
                nc.gpsimd.tensor_mul(vh[:], vh[:], cN[:])
                nc.gpsimd.tensor_add(vh[:], vh[:], vtmp[:])

        def emit_attn_qt(bb, qt, fo_engines):
            qs = bb * N + qt * QT
            pvA = pvp.tile([128, 4, 65], F32, tag="pvA", name=f"pvA{bb}_{qt}")
            pvB = pvp.tile([128, 4, 65], F32, tag="pvB", name=f"pvB{bb}_{qt}")
            nkb = 4 * (qt + 1)

            def emit_pv(kb, att):
                kbg = bb * 16 + kb
                r = kb - 4 * qt
                # One accumulation "zero region" per pv bank: start only on the
                # first matmul (marks whole bank pending-zero; sibling q-block
                # chains then write-through), stop only on the last.
                for h, pv in ((0, pvA), (1, pvB)):
                    for qb in range(max(0, r), 4):
                        nc.tensor.matmul(
                            pv[:, qb, :],
                            att[:, h, qb * 128:(qb + 1) * 128],
                            vsb[:, kbg, h * 65:(h + 1) * 65],
                            start=(kb == 0 and qb == 0),
                            stop=(kb == nkb - 1 and qb == 3),
                            skip_group_check=True)

            # epilogue halves: normalize, transpose, out-project for 2 qbs;
            # emitted as soon as those qbs' PV chains complete so the chain
            # overlaps the remaining S/exp/PV work of this q-tile.
            rc = rcp.tile([128, 2, 4], F32, tag="rc", name=f"rc{bb}_{qt}")
            on = onp.tile([128, 4, 128], BF16, tag="on", name=f"on{bb}_{qt}")
            fo = fop.tile([128, 4, DIM], BF16, tag="fo", name=f"fo{bb}_{qt}")

            def emit_epi_half(half):
                qb2 = slice(2 * half, 2 * half + 2)
                nc.vector.reciprocal(rc[:, 0, qb2], pvA[:, qb2, 64])
                nc.vector.reciprocal(rc[:, 1, qb2], pvB[:, qb2, 64])
                nc.vector.tensor_mul(on[:, qb2, 0:64], pvA[:, qb2, 0:64],
                                     rc[:, 0, qb2, None].to_broadcast([128, 2, 64]))
                nc.vector.tensor_mul(on[:, qb2, 64:128], pvB[:, qb2, 0:64],
                                     rc[:, 1, qb2, None].to_broadcast([128, 2, 64]))
                ptro = pscr.tile([128, 2 * TT], BF16, tag="scr",
                                 name=f"ptro{bb}_{qt}_{half}")
                for i, qb in enumerate(range(2 * half, 2 * half + 2)):
                    nc.tensor.transpose(ptro[:, i * 128:(i + 1) * 128],
                                        on[:, qb, :], ident[:])
                ot = otp.tile([128, 256], BF16, tag="ot", name=f"ot{bb}_{qt}_{half}")
                nc.vector.tensor_copy(ot[:], ptro[:, 0:256])
                for i, qb in enumerate(range(2 * half, 2 * half + 2)):
                    for nn in range(2):
                        po = pscr.tile([128, TT], F32, tag="scr",
                                       name=f"po{bb}_{qt}_{qb}_{nn}")
                        nc.tensor.matmul(po[:], ot[:, i * 128:(i + 1) * 128],
                                         wout_sb[:, nn * 512:(nn + 1) * 512],
                                         start=True, stop=True)
                        eng = fo_engines[(qb * 2 + nn) % len(fo_engines)]
                        eng.tensor_copy(fo[:, qb, nn * 512:(nn + 1) * 512], po[:])

            def emit_store(half):
                nc.sync.dma_start(
                    out[qs + half * 256:qs + (half + 1) * 256, :]
                    .rearrange("(qb p) c -> p qb c", p=128),
                    fo[:, 2 * half:2 * half + 2, :])

            prev = None
            for kb in range(nkb):
                ks = bb * N + kb * 128
                r = kb - 4 * qt
                c0 = 128 * r if r > 0 else 0
                stp = psst.tile([128, 2, QT], F32, tag="st", name=f"st{bb}_{qt}_{kb}")
                for h in range(2):
                    nc.tensor.matmul(
                        stp[:, h, c0:QT],
                        kT[h * 64:(h + 1) * 64, ks:ks + 128],
                        qT[h * 64:(h + 1) * 64, qs + c0:qs + QT],
                        start=True, stop=True)
                att = attp.tile([128, 2, QT], BF16, tag="att", name=f"att{bb}_{qt}_{kb}")
                nc.scalar.activation(att[:, :, c0:QT], stp[:, :, c0:QT],
                                     func=EXP, scale=SCALE)
                if r >= 0:
                    # only the diagonal 128x128 sub-block is ambiguous
                    nc.gpsimd.tensor_mul(
                        att[:, :, c0:c0 + 128], att[:, :, c0:c0 + 128],
                        tri[:, None, :].to_broadcast([128, 2, 128]))
                if prev is not None:
                    emit_pv(*prev)
                    if prev[0] == 4 * qt + 1:
                        emit_epi_half(0)
                        emit_store(0)
                prev = (kb, att)
            emit_pv(*prev)
            emit_epi_half(1)
            emit_store(1)

        for _rep in range(reps):
            # DMA resource is FIFO in emission order: x tiles prefetch first,
            # tables slot in where their consumers can wait, stores trail.
            xts = [bigp.tile([128, KI, TT], BF16, tag="xt", name=f"xt{tt}")
                   for tt in range(NTT)]
            nc.sync.dma_start(xts[0][:, 0:2, :], x_r[:, 0:2, 0:TT])
            nc.sync.dma_start(wcat_sb[:, 0, :, :], wq_t)
            nc.sync.dma_start(xts[0][:, 2:KI, :], x_r[:, 2:KI, 0:TT])
            nc.sync.dma_start(wcat_sb[:, 1:3, :, :], wkv)
            nc.sync.dma_start(pmat_sb[:], pmat)
            nc.sync.dma_start(blob_sb[:, 0:O_WOUT], blob[:, 0:O_WOUT])
            nc.sync.dma_start(xts[1][:], x_r[:, :, TT:2 * TT])
            nc.sync.dma_start(blob_sb[:, O_WOUT:], blob[:, O_WOUT:])
            for tt in range(2, NTT):
                nc.sync.dma_start(xts[tt][:], x_r[:, :, tt * TT:(tt + 1) * TT])
            make_identity(nc, ident)
            nc.vector.memset(vsb[:, :, 64:65], 1.0)
            nc.vector.memset(vsb[:, :, 129:130], 1.0)
            fo_eng = (nc.vector, _ScalarCopy(nc), nc.vector)
            for tt in range(4):
                emit_qkv_tile(tt, xts[tt])
            for qt in range(4):
                emit_qkv_tile(4 + qt, xts[4 + qt])
                emit_attn_qt(0, qt, fo_eng)
            for qt in (3, 2, 1, 0):
                emit_attn_qt(1, qt, fo_eng)

    nc.compile()
    return nc


def _prep_inputs(x, rotary_pos_emb, Wq, Wk, Wv, Wout):
    import ml_dtypes
    bf16 = ml_dtypes.bfloat16

    xT = np.ascontiguousarray(x.reshape(T, DIM).T)            # [DIM, T]
    x_r = np.ascontiguousarray(
        xT.reshape(KI, 128, T).transpose(1, 0, 2)).astype(bf16)

    cos = np.cos(rotary_pos_emb).astype(np.float32)           # [N, L]
    sin = np.sin(rotary_pos_emb).astype(np.float32)
    sin_signed = np.concatenate([-sin[:, :16], sin[:, 16:]], axis=1)

    cosP = np.ones((128, N), np.float32)
    sinP = np.zeros((128, N), np.float32)
    cosP[0:32] = cos.T
    cosP[64:96] = cos.T
    sinP[0:32] = sin.T
    sinP[64:96] = sin.T

    cosN = cos.reshape(16, 128, L).transpose(1, 0, 2).reshape(128, 512)
    sinN = sin_signed.reshape(16, 128, L).transpose(1, 0, 2).reshape(128, 512)

    p_idx = np.arange(128)[:, None]
    c_idx = np.arange(128)[None, :]
    tri = (c_idx - p_idx >= 0).astype(np.float32)

    pmat = np.zeros((128, 128), np.float32)
    for g in (0, 64):
        for j in range(16):
            pmat[g + j + 16, g + j] = -1.0
            pmat[g + j, g + j + 16] = 1.0

    in_maps = []
    for c in range(NCORES):
        sl = slice(c * CPC, (c + 1) * CPC)
        wq = Wq[:, sl].reshape(KI, 128, 128).transpose(1, 0, 2)
        wk = Wk[:, sl].reshape(KI, 128, 128).transpose(1, 0, 2)
        wv = Wv[:, sl].reshape(KI, 128, 128).transpose(1, 0, 2)
        wkv = np.stack([wk, wv], axis=1)  # [128, 2, KI, 128]
        blob = np.zeros((128, BLOB_COLS), np.float32)
        blob[:, O_COSP:O_COSP + N] = cosP
        blob[:, O_SINP:O_SINP + N] = sinP
        blob[:, O_COSN:O_COSN + 512] = cosN
        blob[:, O_SINN:O_SINN + 512] = sinN
        blob[:, O_MASK:O_MASK + 128] = tri
        blob[:, O_WOUT:O_WOUT + DIM] = Wout[sl, :]
        in_maps.append({
            "x_r": x_r,
            "wq_t": np.ascontiguousarray(wq).astype(bf16),
            "wkv": np.ascontiguousarray(wkv).astype(bf16),
            "pmat": pmat.astype(bf16),
            "blob": blob.astype(bf16),
        })
    return in_maps


def kernel(x, rotary_pos_emb, Wq, Wk, Wv, Wout):
    from concourse.bass_utils import run_bass_kernel_spmd

    if "nc" not in _CACHE:
        _CACHE["nc"] = _build_program()
    nc = _CACHE["nc"]

    in_maps = _prep_inputs(np.asarray(x, dtype=np.float32),
                           np.asarray(rotary_pos_emb, dtype=np.float32),
                           np.asarray(Wq, dtype=np.float32),
                           np.asarray(Wk, dtype=np.float32),
                           np.asarray(Wv, dtype=np.float32),
                           np.asarray(Wout, dtype=np.float32))
    res = run_bass_kernel_spmd(nc, in_maps, list(range(NCORES)))
    partial = np.stack([np.asarray(res.results[c]["out"], dtype=np.float32)
                        for c in range(NCORES)])
    full = partial.sum(axis=0).reshape(B, N, DIM).astype(np.float32)
    _CACHE["last_exec_time_ns"] = res.exec_time_ns
    return full
